# revision 39
# baseline (speedup 1.0000x reference)
"""Causal self-attention (B=4, T=2048, D=1024, H=16) on 8 Trainium2 cores.

Sharding: core m = (batch b=m//2, head-group g=m%2 of 8 heads) — data parallel
over batch, tensor parallel over heads.  No device collectives at all: the
projection is row-parallel (each core multiplies its own 512 y-channels by its
512-row slice of w_proj over all 2048 tokens) and returns a PARTIAL [2048,
1024] fp16 output; the host sums each core pair's partials while unsharding.

Precision: all matmul operands are bf16 (x, w, q, k, v, exp(S), y); psum
accumulation f32; output partials fp16.  fp8 was tried and rejected:
quantization noise in a dot product of independent terms does not average
down (signal and noise both grow as sqrt(N)), so ANY fp8 operand costs 2-4%
output RMS vs the 2e-2 gate.  Measured rel err: ~4e-3.

Two TRN2 hardware costs (unmodeled by the cost model, ~500 ns–7 us each)
shape this kernel:
 1. gpsimd Q7 *library reloads* (~7 us) fire whenever consecutive Pool ops
    need different libraries.  All per-group elementwise work (causal masks)
    runs on DVE; the Pool engine executes ONLY partition_broadcast (library
    `attn`), and mask/weight setup is hoisted out of the iteration loop.
 2. PE tiled/full *mode switches* (~465 ns) fire when tile_position matmuls
    alternate with full-128 ones.  K^T is therefore stored zero-padded per
    head half (kt_z: head 2hp+i occupies partition rows 64i..64i+63, zeros
    elsewhere) so score matmuls are full-128 stationaries — mathematically
    identical, same moving-row count, zero mode switches.

Loop-invariant weights (w_qkv, w_proj) and the mask constant load once and
stay SBUF-resident across iterations; kt_z has two iteration-parity slots
and v/x chunk pools are double-buffered so iteration i+1's production only
WAR-waits on iteration i-1's readers; iteration i prefetches i+1's first
x chunk.  Output staging is fp16 (halves out-DMA).

Schedule: per 512-token chunk j, attention runs as 8 (head-pair, head) *
(512-token) groups.  Each group emits its score matmuls eagerly (diagonal:
a 2-bank ragged pack + one 256-wide tile; off-diagonal: 2-bank kt pairs),
and defers its last attn@V matmuls + normalize behind the NEXT group's
score matmuls (attn@V trails exp by ODR_TRAIL kt pairs).  PSUM score banks
are freed by the exp read itself, so 2 rotating pair-slots suffice.
Next-chunk QKV production is interleaved one piece per group via hooks,
and v_chunk(j+1) covers the last group's normalize latency before the
chunk-j projection.  Causal masking multiplies the diagonal 128x128 blocks
by a lower-tri mask on DVE after exp; a ones-column in V yields softmax
denominators for free.

Measured: 728.6 us (session start) -> ~250-270 us; PE busy roofline for
this algorithm is ~235 us (bf16, scores/attn@V at the Dh=64 50% ceiling).
"""

import numpy as np

import concourse.bass as bass
import concourse.mybir as mybir
import concourse.tile as tile
from concourse import bacc
from concourse.bass_utils import run_bass_kernel_spmd

F32 = mybir.dt.float32
F16 = mybir.dt.float16
BF = mybir.dt.bfloat16
F8 = mybir.dt.float8e4

# Problem constants (per spec; hardcoded).
B, T, D, H = 4, 2048, 1024, 16
DH = 64                      # head dim
N_CORES = 8
HC = H // 2                  # heads per core = 8
HP = HC // 2                 # head pairs per core = 4
DK = D // 128                # model-dim contraction tiles = 8
KP = DK // 2                 # DoubleRow contraction pairs = 4
TT = T // 128                # token tiles of 128 = 16
TC = T // 512                # token chunks of 512 = 4
SCALE = 1.0 / 8.0            # 1/sqrt(DH)
EBIAS = -1.0                 # exp bias, guards fp8e4m3 overflow/underflow
WS = 32.0                    # host-side weight scale (w ~ N(0,1/32^2))
NO_COLLECTIVE = False        # kept for tooling compat; kernel has none
DEBUG = False
ODR_TRAIL = 2                # attn@V trails exp by this many kt pairs


def build_kernel(iters=1, parts=("qkv", "attn", "proj")):
    nc = bacc.Bacc("TRN2", target_bir_lowering=False, debug=False,
                   num_devices=N_CORES)

    x_bT = nc.dram_tensor("x_bT", [D, T], BF, kind="ExternalInput").ap()
    w_qkv_my = nc.dram_tensor("w_qkv_my", [D, 3 * 512], BF,
                              kind="ExternalInput").ap()
    w_proj_my = nc.dram_tensor("w_proj_my", [512, D], BF,
                               kind="ExternalInput").ap()
    out = nc.dram_tensor("out", [T, D], F16, kind="ExternalOutput").ap()

    with tile.TileContext(nc) as tc:
        from contextlib import ExitStack
        with ExitStack() as stack:
            shared = _setup_shared(tc, stack, w_qkv_my, w_proj_my)
            for _ in range(iters):
                _emit(tc, x_bT, out, shared, parts=parts)
            for piece in shared.get("proj_prev") or []:
                piece()          # last iteration's deferred projection

    nc.compile()
    return nc


def _setup_shared(tc, stack, w_qkv_my, w_proj_my):
    """Loop-invariant setup, emitted once: causal-mask constant and the
    resident weight tiles (+ their DMAs).  Keeping these out of the
    iteration loop removes 4 MB of DMA per call and all per-iteration
    gpsimd library traffic for mask construction."""
    nc = tc.nc

    const = stack.enter_context(tc.tile_pool(name="const", bufs=1))
    mask_f32 = const.tile([128, 128], F32)
    nc.gpsimd.memset(mask_f32[:], 1.0)
    nc.gpsimd.affine_select(
        out=mask_f32[:], in_=mask_f32[:],
        compare_op=mybir.AluOpType.is_ge,
        fill=0.0, base=0,
        pattern=[[1, 128]],       # + qq
        channel_multiplier=-1,    # - kk
    )
    mask = const.tile([128, 128], BF)
    nc.gpsimd.tensor_copy(mask[:], mask_f32[:])

    wpool = stack.enter_context(tc.tile_pool(name="weights", bufs=1))
    wq_sb = wpool.tile([128, DK, 512], BF, tag="wq")
    wk_sb = wpool.tile([128, DK, 512], BF, tag="wk")
    wv_sb = wpool.tile([128, DK, 512], BF, tag="wv")
    wp_sb = wpool.tile([128, HP, D], BF, tag="wp")
    w_re = w_qkv_my.rearrange("(o p) f -> p o f", p=128)
    nc.sync.dma_start(wq_sb[:], w_re[:, :, 0:512])
    nc.sync.dma_start(wk_sb[:], w_re[:, :, 512:1024])
    nc.sync.dma_start(wv_sb[:], w_re[:, :, 1024:1536])
    nc.sync.dma_start(wp_sb[:],
                      w_proj_my.rearrange("(o p) f -> p o f", p=128))

    # K^T stored zero-padded per head half: kt_z[:, c, i, hp, t] holds head
    # 2hp+i's key dims in partition rows 64i..64i+63 and ZEROS in the other
    # 64 rows, so score matmuls can use full-128 stationaries (no
    # tile_position => no PE tiled/full mode switches, ~465 ns each on HW).
    # c is an iteration-parity slot: consecutive iterations alternate, so
    # iteration i+1's K writes only WAR-wait on iteration i-1's readers.
    kt_z = wpool.tile([128, 2, 2, HP, T], BF, tag="ktz")
    for c in range(2):
        nc.vector.memset(kt_z[64:128, c, 0], 0.0)
        nc.vector.memset(kt_z[0:64, c, 1], 0.0)
    # All working pools live at shared scope: closing a pool inside the
    # iteration loop emits per-iteration Drain barriers (~2-4 us each).
    # Tiles are still allocated per iteration; slots just rotate.
    pools = dict(
        xtp=stack.enter_context(tc.tile_pool(name="xt", bufs=2)),
        qtp=stack.enter_context(tc.tile_pool(name="qt", bufs=2)),
        ybp=stack.enter_context(tc.tile_pool(name="yb", bufs=2)),
        epp=stack.enter_context(tc.tile_pool(name="ep", bufs=4)),
        edp=stack.enter_context(tc.tile_pool(name="ed", bufs=2)),
        nrmp=stack.enter_context(tc.tile_pool(name="nrm", bufs=2)),
        ostp=stack.enter_context(tc.tile_pool(name="ost", bufs=2)),
        psp=stack.enter_context(tc.tile_pool(name="ps", bufs=1,
                                             space="PSUM")),
        persist=stack.enter_context(tc.tile_pool(name="persist", bufs=2)),
    )
    return dict(mask=mask, wq=wq_sb, wk=wk_sb, wv=wv_sb, wp=wp_sb,
                kt=kt_z, it=[0], xt_next=[None], **pools)


def _emit(tc, x_bT, out, shared, parts=("qkv", "attn", "proj")):
    nc = tc.nc

    mask = shared["mask"]
    wq_sb, wk_sb, wv_sb, wp_sb = (shared["wq"], shared["wk"], shared["wv"],
                                  shared["wp"])
    parity = shared["it"][0] & 1
    shared["it"][0] += 1
    kt_z = shared["kt"][:, parity]

    v_sb = shared["persist"].tile([128, TT, HC, 65], BF, tag="v")

    x_re = x_bT.rearrange("(o p) t -> p o t", p=128)

    # col 64 = 1.0 supplies softmax denominators.
    nc.vector.memset(v_sb[:, :, :, 64:65], 1.0)

    xtp, qtp, ybp, epp, edp, nrmp, ostp, psp = (
        shared["xtp"], shared["qtp"], shared["ybp"], shared["epp"],
        shared["edp"], shared["nrmp"], shared["ostp"], shared["psp"])
    prev_proj = shared.get("proj_prev") or []
    shared["proj_prev"] = []
    if True:
        pending = []             # deferred closures (last O + norm per group)

        def flush():
            while pending:
                pending.pop(0)()

        def x_dma(jn):
            xt = xtp.tile([128, DK, 512], BF, tag="xt", name="xt")
            nc.sync.dma_start(xt[:], x_re[:, :, jn * 512:(jn + 1) * 512])
            return xt

        def qkv_chunk_dma(jn):
            return x_dma(jn), qtp.tile([128, HP, 512], BF, tag="qt",
                                       name="qt")

        def qk_piece(jn, xt, qt, p):
            # one of 8 per-chunk QK productions: p = 2*hp' + (0:q, 1:k)
            hpn, which = p // 2, p % 2
            wsb = wq_sb if which == 0 else wk_sb
            ps = psp.tile([128, 512], F32, tag="mm512", name="qkps", bufs=2)
            for kc in range(DK):
                nc.tensor.matmul(
                    ps[:],
                    lhsT=wsb[:, kc, hpn * 128:(hpn + 1) * 128],
                    rhs=xt[:, kc, :],
                    start=(kc == 0), stop=(kc == DK - 1))
            if which == 0:
                nc.vector.tensor_copy(qt[:, hpn, :], ps[:])
            else:
                # split halves into the zero-padded per-head K tiles
                sl = slice(jn * 512, (jn + 1) * 512)
                nc.vector.tensor_copy(kt_z[0:64, 0, hpn, sl], ps[0:64, :])
                nc.vector.tensor_copy(kt_z[64:128, 1, hpn, sl],
                                      ps[64:128, :])

        def v_chunk(jn, xt):
            for tt in range(4):
                ps = psp.tile([128, 512], F32, tag="mm512", name="vps",
                              bufs=2)
                for kc in range(DK):
                    nc.tensor.matmul(
                        ps[:],
                        lhsT=xt[:, kc, tt * 128:(tt + 1) * 128],
                        rhs=wv_sb[:, kc, :],
                        start=(kc == 0), stop=(kc == DK - 1))
                nc.vector.tensor_copy(
                    v_sb[:, jn * 4 + tt, :, 0:64],
                    ps[:].rearrange("p (h d) -> p h d", h=HC))

        def attn_group(j, hp, i, qt, yb, hook=None):
            """One (head-pair, band) attention group, software-pipelined:
            emits its S work eagerly, defers its last O + normalize into
            `pending` so the next group's S matmuls slot in front of them.
            `hook` (interleaved non-Act work, e.g. next-chunk QK) is emitted
            after the S-phase so it never delays the next exp."""
            h = 2 * hp + i
            P = 2 * j
            o_ps = psp.tile([128, 512], F32, tag=f"ops{i}",
                            name=f"ops{i}", bufs=1)
            qv = qt[:, hp, :]
            kv = kt_z[:, i, hp, :]

            # diagonal scores: dk0|dk1|dk3 packed into 2 banks, dk2 separate
            pack = psp.tile([128, 1024], F32, tag="pair", name="pack", bufs=2)
            s256 = psp.tile([128, 512], F32, tag="mm512", name="s256", bufs=2)
            diag = ((0, 0, slice(0, 512)), (1, 128, slice(512, 896)),
                    (3, 384, slice(896, 1024)))
            for dk, o, sl in diag:
                kt_i = 4 * j + dk
                nc.tensor.matmul(
                    pack[:, sl], lhsT=kv[:, kt_i * 128:(kt_i + 1) * 128],
                    rhs=qv[:, o:512], start=True, stop=True)
            kt_i = 4 * j + 2
            nc.tensor.matmul(
                s256[:, 0:256], lhsT=kv[:, kt_i * 128:(kt_i + 1) * 128],
                rhs=qv[:, 256:512], start=True, stop=True)

            flush()          # previous group's deferred last-O + normalize
            if hook is not None:
                hook()

            e_pack = edp.tile([128, 1024], BF, tag="epk", name="e_pack")
            # halves: attn@V's first matmul only needs cols 0:512, so it can
            # start while the Act engine still works on the second half
            nc.scalar.activation(e_pack[:, 0:512], pack[:, 0:512],
                                 mybir.ActivationFunctionType.Exp,
                                 scale=SCALE)
            nc.scalar.activation(e_pack[:, 512:1024], pack[:, 512:1024],
                                 mybir.ActivationFunctionType.Exp,
                                 scale=SCALE)
            e_s = edp.tile([128, 256], BF, tag="es", name="e_s")
            nc.scalar.activation(e_s[:], s256[:, 0:256],
                                 mybir.ActivationFunctionType.Exp,
                                 scale=SCALE)
            for eap in (e_pack[:, 0:128], e_pack[:, 512:640],
                        e_pack[:, 896:1024], e_s[:, 0:128]):
                nc.vector.tensor_tensor(eap, eap, mask[:],
                                        mybir.AluOpType.mult)
            for rhs, dk, o in ((e_pack[:, 0:512], 0, 0),
                               (e_pack[:, 512:896], 1, 128),
                               (e_pack[:, 896:1024], 3, 384),
                               (e_s[:, 0:256], 2, 256)):
                kt_i = 4 * j + dk
                nc.tensor.matmul(
                    o_ps[0:65, o:512], lhsT=v_sb[:, kt_i, h, 0:65],
                    rhs=rhs, start=(dk == 0), stop=(j == 0 and dk == 2),
                    skip_group_check=True)

            # off-diagonal kt pairs: S bf16 -> exp pair -> attn@V bf16.
            # O(kp) is emitted ODR_TRAIL pairs later; the last O is deferred.
            e_pairs = []
            for kp in range(P):
                spair = psp.tile([128, 2, 512], F32, tag="pair",
                                 name="spair", bufs=2)
                for u in range(2):
                    kt_i = 2 * kp + u
                    nc.tensor.matmul(
                        spair[:, u, :],
                        lhsT=kv[:, kt_i * 128:(kt_i + 1) * 128],
                        rhs=qv[:], start=True, stop=True)
                e_pair = epp.tile([128, 2, 512], BF, tag="epr",
                                  name="e_pair")
                nc.scalar.activation(e_pair[:], spair[:],
                                     mybir.ActivationFunctionType.Exp,
                                     scale=SCALE)
                e_pairs.append(e_pair)
                if kp >= ODR_TRAIL:
                    emit_odr(j, h, o_ps, e_pairs[kp - ODR_TRAIL],
                             kp - ODR_TRAIL)

            def tail():
                for kpt in range(max(P - ODR_TRAIL, 0), P):
                    emit_odr(j, h, o_ps, e_pairs[kpt], kpt)
                # normalize: y = num * (1/den)
                rec = nrmp.tile([1, 512], F32, tag="rec", name="rec")
                nc.vector.reciprocal(rec[:], o_ps[64:65, :])
                rec_b = nrmp.tile([64, 512], F32, tag="recb", name="rec_b")
                nc.gpsimd.partition_broadcast(rec_b[:], rec[:])
                nc.vector.tensor_tensor(
                    yb[64 * i:64 * (i + 1), hp, :],
                    o_ps[0:64, :], rec_b[:], mybir.AluOpType.mult)
            pending.append(tail)

        def emit_odr(j, h, o_ps, e_pair, kp):
            for u in range(2):
                nc.tensor.matmul(
                    o_ps[0:65, :], lhsT=v_sb[:, 2 * kp + u, h, 0:65],
                    rhs=e_pair[:, u, :], start=False,
                    stop=(kp == 2 * j - 1 and u == 1),
                    skip_group_check=True)

        def proj_piece(j, yb, ost, tt, nn):
            # one (tt, nn) projection tile: 4-matmul psum chain + fp16 copy;
            # the nn=1 piece also fires the row DMA (deps cover both copies)
            ps = psp.tile([128, 512], F32, tag="mm512", name="pps",
                          bufs=2)
            for hp2 in range(HP):
                nc.tensor.matmul(
                    ps[:],
                    lhsT=yb[:, hp2, tt * 128:(tt + 1) * 128],
                    rhs=wp_sb[:, hp2, nn * 512:(nn + 1) * 512],
                    start=(hp2 == 0), stop=(hp2 == HP - 1))
            nc.vector.tensor_copy(
                ost[:, tt, nn * 512:(nn + 1) * 512], ps[:])
            if nn == 1:
                nc.sync.dma_start(
                    out[j * 512 + tt * 128:j * 512 + (tt + 1) * 128, :],
                    ost[:, tt, :])

        def proj_block(j, yb):
            # fp16 staging halves the out-DMA and SBUF cost; partial sums
            # are ~O(1) so fp16 rounding (~5e-4 rel) is far inside the gate
            ost = ostp.tile([128, 4, D], F16, tag="ost", name="ost")
            for tt in range(4):
                for nn in range(2):
                    proj_piece(j, yb, ost, tt, nn)

        def make_proj_hooks(j, yb):
            # defer chunk j's projection into the NEXT chunk's 8 group hooks
            # so the last chunk's groups (which have no QKV to produce) still
            # have PE filler covering the exp->attn@V latency
            ost = ostp.tile([128, 4, D], F16, tag="ost", name="ost")
            return [
                (lambda tt=tt, nn=nn: proj_piece(j, yb, ost, tt, nn))
                for tt in range(4) for nn in range(2)
            ]

        # ---- chunk 0 bootstrap: only hp=0's q/k + V gate the first group --
        if shared["xt_next"][0] is not None:
            xt = shared["xt_next"][0]         # prefetched by previous iter
            shared["xt_next"][0] = None
            qt = qtp.tile([128, HP, 512], BF, tag="qt", name="qt")
        else:
            xt, qt = qkv_chunk_dma(0)
        qk_piece(0, xt, qt, 0)
        qk_piece(0, xt, qt, 1)
        v_chunk(0, xt)

        proj_hooks = []
        for j in range(TC):
            if j < TC - 1:
                xt_n, qt_n = qkv_chunk_dma(j + 1)
            else:
                shared["xt_next"][0] = x_dma(0)   # next iteration's chunk 0
            yb = ybp.tile([128, HP, 512], BF, tag="yb", name="yb")
            for hp in range(HP):
                for i in range(2):
                    g_idx = 2 * hp + i
                    if j == 0:
                        # finish chunk 0's own QK one group ahead of use,
                        # plus chunk 1's piece, plus one piece of the
                        # PREVIOUS iteration's deferred final projection
                        def hook(g=g_idx):
                            if g + 2 < 8:
                                qk_piece(0, xt, qt, g + 2)
                            qk_piece(1, xt_n, qt_n, g)
                            if g < len(prev_proj):
                                prev_proj[g]()
                    elif j < TC - 1:
                        hook = (lambda g=g_idx: qk_piece(j + 1, xt_n, qt_n, g))
                    elif proj_hooks:
                        hook = proj_hooks[g_idx]  # chunk TC-2's projection
                    else:
                        hook = None
                    if "attn" in parts:
                        attn_group(j, hp, i, qt, yb, hook=hook)
                    elif hook is not None:
                        hook()
            if j < TC - 1:
                v_chunk(j + 1, xt_n)
                xt, qt = xt_n, qt_n
            flush()
            if "proj" in parts:
                if j == TC - 2:
                    proj_hooks = make_proj_hooks(j, yb)
                elif j == TC - 1:
                    # defer across the iteration boundary; build_kernel
                    # emits any leftovers after the last iteration
                    shared["proj_prev"] = make_proj_hooks(j, yb)
                else:
                    proj_block(j, yb)




_NC_CACHE = None
LAST_RESULT = None


def _prep_in_maps(x, w_qkv, w_proj):
    import ml_dtypes
    bf16 = ml_dtypes.bfloat16
    in_maps = []
    for m in range(N_CORES):
        b, g = m // 2, m % 2
        w_my = np.concatenate(
            [w_qkv[:, g * 512:(g + 1) * 512],
             w_qkv[:, 1024 + g * 512:1024 + (g + 1) * 512],
             w_qkv[:, 2048 + g * 512:2048 + (g + 1) * 512]], axis=1)
        in_maps.append({
            "x_bT": np.ascontiguousarray(x[b].T).astype(bf16),
            "w_qkv_my": np.ascontiguousarray(w_my).astype(bf16),
            "w_proj_my": np.ascontiguousarray(
                w_proj[g * 512:(g + 1) * 512, :]).astype(bf16),
        })
    return in_maps


def kernel(x, w_qkv, w_proj):
    global _NC_CACHE, LAST_RESULT
    x = np.asarray(x, dtype=np.float32)
    w_qkv = np.asarray(w_qkv, dtype=np.float32)
    w_proj = np.asarray(w_proj, dtype=np.float32)

    if _NC_CACHE is None:
        _NC_CACHE = build_kernel()
    nc = _NC_CACHE

    in_maps = _prep_in_maps(x, w_qkv, w_proj)
    res = run_bass_kernel_spmd(nc, in_maps, core_ids=list(range(N_CORES)))
    LAST_RESULT = res
    out = np.empty((B, T, D), dtype=np.float32)
    for b in range(B):
        out[b] = (res.results[2 * b]["out"].astype(np.float32)
                  + res.results[2 * b + 1]["out"].astype(np.float32))
    return out



# revision 42
# speedup vs baseline: 1.1054x; 1.1054x over previous
"""Causal self-attention (B=4, T=2048, D=1024, H=16) on 8 Trainium2 cores.

Sharding: core m = (batch b=m//2, head-group g=m%2 of 8 heads) — data parallel
over batch, tensor parallel over heads.  No device collectives at all: the
projection is row-parallel (each core multiplies its own 512 y-channels by its
512-row slice of w_proj over all 2048 tokens) and returns a PARTIAL [2048,
1024] fp16 output; the host sums each core pair's partials while unsharding.

Precision: all matmul operands are bf16 (x, w, q, k, v, exp(S), y); psum
accumulation f32; output partials fp16.  fp8 was tried and rejected:
quantization noise in a dot product of independent terms does not average
down (signal and noise both grow as sqrt(N)), so ANY fp8 operand costs 2-4%
output RMS vs the 2e-2 gate.  Measured rel err: ~4e-3.

Two TRN2 hardware costs (unmodeled by the cost model, ~500 ns–7 us each)
shape this kernel:
 1. gpsimd Q7 *library reloads* (~7 us) fire whenever consecutive Pool ops
    need different libraries.  All per-group elementwise work (causal masks)
    runs on DVE; the Pool engine executes ONLY partition_broadcast (library
    `attn`), and mask/weight setup is hoisted out of the iteration loop.
 2. PE tiled/full *mode switches* (~465 ns) fire when tile_position matmuls
    alternate with full-128 ones.  K^T is therefore stored zero-padded per
    head half (kt_z: head 2hp+i occupies partition rows 64i..64i+63, zeros
    elsewhere) so score matmuls are full-128 stationaries — mathematically
    identical, same moving-row count, zero mode switches.

Loop-invariant weights (w_qkv, w_proj) and the mask constant load once and
stay SBUF-resident across iterations; kt_z has two iteration-parity slots
and v/x chunk pools are double-buffered so iteration i+1's production only
WAR-waits on iteration i-1's readers; iteration i prefetches i+1's first
x chunk.  Output staging is fp16 (halves out-DMA).

Schedule: per 512-token chunk j, attention runs as 8 (head-pair, head) *
(512-token) groups.  Each group emits its score matmuls eagerly (diagonal:
a 2-bank ragged pack + one 256-wide tile; off-diagonal: 2-bank kt pairs),
and defers its last attn@V matmuls + normalize behind the NEXT group's
score matmuls (attn@V trails exp by ODR_TRAIL kt pairs).  PSUM score banks
are freed by the exp read itself, so 2 rotating pair-slots suffice.
Every group carries PE "hook" filler covering the exp->attn@V latency:
chunks 0..TC-2 produce the next chunk's QKV one piece per group; chunk
TC-1's groups instead run chunk TC-2's projection pieces, and the FINAL
chunk's projection defers across the iteration boundary into the next
iteration's chunk-0 hooks (build_kernel emits the leftovers after the
last iteration).  All tile pools live at shared scope — closing a pool
per iteration emits multi-us Drain barriers.  The diag exp is split in
halves so attn@V's first matmul waits only on cols 0:512.  Causal masking
multiplies the diagonal 128x128 blocks by a lower-tri mask on DVE after
exp; a ones-column in V yields softmax denominators for free.

Measured: 728.6 us (session start) -> ~250 us; TimelineSim marginal is
241.6 us/iter = its PE.ENGINE busy time exactly (zero modeled stalls);
the PE roofline is set by bf16 with scores/attn@V at the Dh=64 50%
out-partition ceiling.
"""

import numpy as np

import concourse.bass as bass
import concourse.mybir as mybir
import concourse.tile as tile
from concourse import bacc
from concourse.bass_utils import run_bass_kernel_spmd

F32 = mybir.dt.float32
F16 = mybir.dt.float16
BF = mybir.dt.bfloat16
F8 = mybir.dt.float8e4

# Problem constants (per spec; hardcoded).
B, T, D, H = 4, 2048, 1024, 16
DH = 64                      # head dim
N_CORES = 8
HC = H // 2                  # heads per core = 8
HP = HC // 2                 # head pairs per core = 4
DK = D // 128                # model-dim contraction tiles = 8
KP = DK // 2                 # DoubleRow contraction pairs = 4
TT = T // 128                # token tiles of 128 = 16
TC = T // 512                # token chunks of 512 = 4
SCALE = 1.0 / 8.0            # 1/sqrt(DH)
EBIAS = -1.0                 # exp bias, guards fp8e4m3 overflow/underflow
WS = 32.0                    # host-side weight scale (w ~ N(0,1/32^2))
NO_COLLECTIVE = False        # kept for tooling compat; kernel has none
DEBUG = False
ODR_TRAIL = 2                # attn@V trails exp by this many kt pairs


def build_kernel(iters=1, parts=("qkv", "attn", "proj")):
    nc = bacc.Bacc("TRN2", target_bir_lowering=False, debug=False,
                   num_devices=N_CORES)

    x_bT = nc.dram_tensor("x_bT", [D, T], BF, kind="ExternalInput").ap()
    w_qkv_my = nc.dram_tensor("w_qkv_my", [D, 3 * 512], BF,
                              kind="ExternalInput").ap()
    w_proj_my = nc.dram_tensor("w_proj_my", [512, D], BF,
                               kind="ExternalInput").ap()
    out = nc.dram_tensor("out", [T, D], F16, kind="ExternalOutput").ap()

    with tile.TileContext(nc) as tc:
        from contextlib import ExitStack
        with ExitStack() as stack:
            shared = _setup_shared(tc, stack, w_qkv_my, w_proj_my)
            for _ in range(iters):
                _emit(tc, x_bT, out, shared, parts=parts)
            for piece in shared.get("proj_prev") or []:
                piece()          # last iteration's deferred projection

    nc.compile()
    return nc


def _setup_shared(tc, stack, w_qkv_my, w_proj_my):
    """Loop-invariant setup, emitted once: causal-mask constant and the
    resident weight tiles (+ their DMAs).  Keeping these out of the
    iteration loop removes 4 MB of DMA per call and all per-iteration
    gpsimd library traffic for mask construction."""
    nc = tc.nc

    const = stack.enter_context(tc.tile_pool(name="const", bufs=1))
    mask_f32 = const.tile([128, 128], F32)
    nc.gpsimd.memset(mask_f32[:], 1.0)
    nc.gpsimd.affine_select(
        out=mask_f32[:], in_=mask_f32[:],
        compare_op=mybir.AluOpType.is_ge,
        fill=0.0, base=0,
        pattern=[[1, 128]],       # + qq
        channel_multiplier=-1,    # - kk
    )
    mask = const.tile([128, 128], BF)
    nc.gpsimd.tensor_copy(mask[:], mask_f32[:])

    wpool = stack.enter_context(tc.tile_pool(name="weights", bufs=1))
    wq_sb = wpool.tile([128, DK, 512], BF, tag="wq")
    wk_sb = wpool.tile([128, DK, 512], BF, tag="wk")
    wv_sb = wpool.tile([128, DK, 512], BF, tag="wv")
    wp_sb = wpool.tile([128, HP, D], BF, tag="wp")
    w_re = w_qkv_my.rearrange("(o p) f -> p o f", p=128)
    nc.sync.dma_start(wq_sb[:], w_re[:, :, 0:512])
    nc.sync.dma_start(wk_sb[:], w_re[:, :, 512:1024])
    nc.sync.dma_start(wv_sb[:], w_re[:, :, 1024:1536])
    nc.sync.dma_start(wp_sb[:],
                      w_proj_my.rearrange("(o p) f -> p o f", p=128))

    # K^T stored zero-padded per head half: kt_z[:, c, i, hp, t] holds head
    # 2hp+i's key dims in partition rows 64i..64i+63 and ZEROS in the other
    # 64 rows, so score matmuls can use full-128 stationaries (no
    # tile_position => no PE tiled/full mode switches, ~465 ns each on HW).
    # c is an iteration-parity slot: consecutive iterations alternate, so
    # iteration i+1's K writes only WAR-wait on iteration i-1's readers.
    kt_z = wpool.tile([128, 2, 2, HP, T], BF, tag="ktz")
    for c in range(2):
        nc.vector.memset(kt_z[64:128, c, 0], 0.0)
        nc.vector.memset(kt_z[0:64, c, 1], 0.0)
    # All working pools live at shared scope: closing a pool inside the
    # iteration loop emits per-iteration Drain barriers (~2-4 us each).
    # Tiles are still allocated per iteration; slots just rotate.
    pools = dict(
        xtp=stack.enter_context(tc.tile_pool(name="xt", bufs=2)),
        qtp=stack.enter_context(tc.tile_pool(name="qt", bufs=2)),
        ybp=stack.enter_context(tc.tile_pool(name="yb", bufs=2)),
        epp=stack.enter_context(tc.tile_pool(name="ep", bufs=4)),
        edp=stack.enter_context(tc.tile_pool(name="ed", bufs=2)),
        nrmp=stack.enter_context(tc.tile_pool(name="nrm", bufs=2)),
        ostp=stack.enter_context(tc.tile_pool(name="ost", bufs=2)),
        psp=stack.enter_context(tc.tile_pool(name="ps", bufs=1,
                                             space="PSUM")),
        persist=stack.enter_context(tc.tile_pool(name="persist", bufs=2)),
    )
    return dict(mask=mask, wq=wq_sb, wk=wk_sb, wv=wv_sb, wp=wp_sb,
                kt=kt_z, it=[0], xt_next=[None], **pools)


def _emit(tc, x_bT, out, shared, parts=("qkv", "attn", "proj")):
    nc = tc.nc

    mask = shared["mask"]
    wq_sb, wk_sb, wv_sb, wp_sb = (shared["wq"], shared["wk"], shared["wv"],
                                  shared["wp"])
    parity = shared["it"][0] & 1
    shared["it"][0] += 1
    kt_z = shared["kt"][:, parity]

    v_sb = shared["persist"].tile([128, TT, HC, 65], BF, tag="v")

    x_re = x_bT.rearrange("(o p) t -> p o t", p=128)

    # col 64 = 1.0 supplies softmax denominators.
    nc.vector.memset(v_sb[:, :, :, 64:65], 1.0)

    xtp, qtp, ybp, epp, edp, nrmp, ostp, psp = (
        shared["xtp"], shared["qtp"], shared["ybp"], shared["epp"],
        shared["edp"], shared["nrmp"], shared["ostp"], shared["psp"])
    prev_proj = shared.get("proj_prev") or []
    shared["proj_prev"] = []
    if True:
        pending = []             # deferred closures (last O + norm per group)

        def flush():
            while pending:
                pending.pop(0)()

        def x_dma(jn):
            xt = xtp.tile([128, DK, 512], BF, tag="xt", name="xt")
            nc.sync.dma_start(xt[:], x_re[:, :, jn * 512:(jn + 1) * 512])
            return xt

        def qkv_chunk_dma(jn):
            return x_dma(jn), qtp.tile([128, HP, 512], BF, tag="qt",
                                       name="qt")

        def qk_piece(jn, xt, qt, p):
            # one of 8 per-chunk QK productions: p = 2*hp' + (0:q, 1:k)
            hpn, which = p // 2, p % 2
            wsb = wq_sb if which == 0 else wk_sb
            ps = psp.tile([128, 512], F32, tag="mm512", name="qkps", bufs=2)
            for kc in range(DK):
                nc.tensor.matmul(
                    ps[:],
                    lhsT=wsb[:, kc, hpn * 128:(hpn + 1) * 128],
                    rhs=xt[:, kc, :],
                    start=(kc == 0), stop=(kc == DK - 1))
            if which == 0:
                nc.vector.tensor_copy(qt[:, hpn, :], ps[:])
            else:
                # split halves into the zero-padded per-head K tiles
                sl = slice(jn * 512, (jn + 1) * 512)
                nc.vector.tensor_copy(kt_z[0:64, 0, hpn, sl], ps[0:64, :])
                nc.vector.tensor_copy(kt_z[64:128, 1, hpn, sl],
                                      ps[64:128, :])

        def v_chunk(jn, xt):
            for tt in range(4):
                ps = psp.tile([128, 512], F32, tag="mm512", name="vps",
                              bufs=2)
                for kc in range(DK):
                    nc.tensor.matmul(
                        ps[:],
                        lhsT=xt[:, kc, tt * 128:(tt + 1) * 128],
                        rhs=wv_sb[:, kc, :],
                        start=(kc == 0), stop=(kc == DK - 1))
                nc.vector.tensor_copy(
                    v_sb[:, jn * 4 + tt, :, 0:64],
                    ps[:].rearrange("p (h d) -> p h d", h=HC))

        def attn_group(j, hp, i, qt, yb, hook=None):
            """One (head-pair, band) attention group, software-pipelined:
            emits its S work eagerly, defers its last O + normalize into
            `pending` so the next group's S matmuls slot in front of them.
            `hook` (interleaved non-Act work, e.g. next-chunk QK) is emitted
            after the S-phase so it never delays the next exp."""
            h = 2 * hp + i
            P = 2 * j
            o_ps = psp.tile([128, 512], F32, tag=f"ops{i}",
                            name=f"ops{i}", bufs=1)
            qv = qt[:, hp, :]
            kv = kt_z[:, i, hp, :]

            # diagonal scores: dk0|dk1|dk3 packed into 2 banks, dk2 separate
            pack = psp.tile([128, 1024], F32, tag="pair", name="pack", bufs=2)
            s256 = psp.tile([128, 512], F32, tag="mm512", name="s256", bufs=2)
            diag = ((0, 0, slice(0, 512)), (1, 128, slice(512, 896)),
                    (3, 384, slice(896, 1024)))
            for dk, o, sl in diag:
                kt_i = 4 * j + dk
                nc.tensor.matmul(
                    pack[:, sl], lhsT=kv[:, kt_i * 128:(kt_i + 1) * 128],
                    rhs=qv[:, o:512], start=True, stop=True)
            kt_i = 4 * j + 2
            nc.tensor.matmul(
                s256[:, 0:256], lhsT=kv[:, kt_i * 128:(kt_i + 1) * 128],
                rhs=qv[:, 256:512], start=True, stop=True)

            flush()          # previous group's deferred last-O + normalize
            if hook is not None:
                hook()

            e_pack = edp.tile([128, 1024], BF, tag="epk", name="e_pack")
            # halves: attn@V's first matmul only needs cols 0:512, so it can
            # start while the Act engine still works on the second half
            nc.scalar.activation(e_pack[:, 0:512], pack[:, 0:512],
                                 mybir.ActivationFunctionType.Exp,
                                 scale=SCALE)
            nc.scalar.activation(e_pack[:, 512:1024], pack[:, 512:1024],
                                 mybir.ActivationFunctionType.Exp,
                                 scale=SCALE)
            e_s = edp.tile([128, 256], BF, tag="es", name="e_s")
            nc.scalar.activation(e_s[:], s256[:, 0:256],
                                 mybir.ActivationFunctionType.Exp,
                                 scale=SCALE)
            # NOTE: psum start/stop accumulation groups are BANK-granular on
            # HW — splitting the diag into two phases with two start=True
            # sub-ranges on this bank corrupts the accumulator (measured
            # rel err 1.2).  Keep the single full-range start on dk0.
            for eap in (e_pack[:, 0:128], e_pack[:, 512:640],
                        e_pack[:, 896:1024], e_s[:, 0:128]):
                nc.vector.tensor_tensor(eap, eap, mask[:],
                                        mybir.AluOpType.mult)
            for rhs, dk, o in ((e_pack[:, 0:512], 0, 0),
                               (e_pack[:, 512:896], 1, 128),
                               (e_pack[:, 896:1024], 3, 384),
                               (e_s[:, 0:256], 2, 256)):
                kt_i = 4 * j + dk
                nc.tensor.matmul(
                    o_ps[0:65, o:512], lhsT=v_sb[:, kt_i, h, 0:65],
                    rhs=rhs, start=(dk == 0), stop=(j == 0 and dk == 2),
                    skip_group_check=True)

            # off-diagonal kt pairs: S bf16 -> exp pair -> attn@V bf16.
            # O(kp) is emitted ODR_TRAIL pairs later; the last O is deferred.
            e_pairs = []
            for kp in range(P):
                spair = psp.tile([128, 2, 512], F32, tag="pair",
                                 name="spair", bufs=2)
                for u in range(2):
                    kt_i = 2 * kp + u
                    nc.tensor.matmul(
                        spair[:, u, :],
                        lhsT=kv[:, kt_i * 128:(kt_i + 1) * 128],
                        rhs=qv[:], start=True, stop=True)
                e_pair = epp.tile([128, 2, 512], BF, tag="epr",
                                  name="e_pair")
                nc.scalar.activation(e_pair[:], spair[:],
                                     mybir.ActivationFunctionType.Exp,
                                     scale=SCALE)
                e_pairs.append(e_pair)
                if kp >= ODR_TRAIL:
                    emit_odr(j, h, o_ps, e_pairs[kp - ODR_TRAIL],
                             kp - ODR_TRAIL)

            def tail():
                for kpt in range(max(P - ODR_TRAIL, 0), P):
                    emit_odr(j, h, o_ps, e_pairs[kpt], kpt)
                # normalize: y = num * (1/den)
                rec = nrmp.tile([1, 512], F32, tag="rec", name="rec")
                nc.vector.reciprocal(rec[:], o_ps[64:65, :])
                rec_b = nrmp.tile([64, 512], F32, tag="recb", name="rec_b")
                nc.gpsimd.partition_broadcast(rec_b[:], rec[:])
                nc.vector.tensor_tensor(
                    yb[64 * i:64 * (i + 1), hp, :],
                    o_ps[0:64, :], rec_b[:], mybir.AluOpType.mult)
            pending.append(tail)

        def emit_odr(j, h, o_ps, e_pair, kp):
            for u in range(2):
                nc.tensor.matmul(
                    o_ps[0:65, :], lhsT=v_sb[:, 2 * kp + u, h, 0:65],
                    rhs=e_pair[:, u, :], start=False,
                    stop=(kp == 2 * j - 1 and u == 1),
                    skip_group_check=True)

        def proj_piece(j, yb, ost, tt, nn):
            # one (tt, nn) projection tile: 4-matmul psum chain + fp16 copy;
            # the nn=1 piece also fires the row DMA (deps cover both copies)
            ps = psp.tile([128, 512], F32, tag="mm512", name="pps",
                          bufs=2)
            for hp2 in range(HP):
                nc.tensor.matmul(
                    ps[:],
                    lhsT=yb[:, hp2, tt * 128:(tt + 1) * 128],
                    rhs=wp_sb[:, hp2, nn * 512:(nn + 1) * 512],
                    start=(hp2 == 0), stop=(hp2 == HP - 1))
            nc.vector.tensor_copy(
                ost[:, tt, nn * 512:(nn + 1) * 512], ps[:])
            if nn == 1:
                nc.sync.dma_start(
                    out[j * 512 + tt * 128:j * 512 + (tt + 1) * 128, :],
                    ost[:, tt, :])

        def proj_block(j, yb):
            # fp16 staging halves the out-DMA and SBUF cost; partial sums
            # are ~O(1) so fp16 rounding (~5e-4 rel) is far inside the gate
            ost = ostp.tile([128, 4, D], F16, tag="ost", name="ost")
            for tt in range(4):
                for nn in range(2):
                    proj_piece(j, yb, ost, tt, nn)

        def make_proj_hooks(j, yb):
            # defer chunk j's projection into the NEXT chunk's 8 group hooks
            # so the last chunk's groups (which have no QKV to produce) still
            # have PE filler covering the exp->attn@V latency
            ost = ostp.tile([128, 4, D], F16, tag="ost", name="ost")
            return [
                (lambda tt=tt, nn=nn: proj_piece(j, yb, ost, tt, nn))
                for tt in range(4) for nn in range(2)
            ]

        # ---- chunk 0 bootstrap: only hp=0's q/k + V gate the first group --
        if shared["xt_next"][0] is not None:
            xt = shared["xt_next"][0]         # prefetched by previous iter
            shared["xt_next"][0] = None
            qt = qtp.tile([128, HP, 512], BF, tag="qt", name="qt")
        else:
            xt, qt = qkv_chunk_dma(0)
        qk_piece(0, xt, qt, 0)
        qk_piece(0, xt, qt, 1)
        v_chunk(0, xt)

        proj_hooks = []
        for j in range(TC):
            if j < TC - 1:
                xt_n, qt_n = qkv_chunk_dma(j + 1)
            else:
                shared["xt_next"][0] = x_dma(0)   # next iteration's chunk 0
            yb = ybp.tile([128, HP, 512], BF, tag="yb", name="yb")
            for hp in range(HP):
                for i in range(2):
                    g_idx = 2 * hp + i
                    if j == 0:
                        # finish chunk 0's own QK one group ahead of use,
                        # plus chunk 1's piece, plus one piece of the
                        # PREVIOUS iteration's deferred final projection
                        def hook(g=g_idx):
                            if g + 2 < 8:
                                qk_piece(0, xt, qt, g + 2)
                            qk_piece(1, xt_n, qt_n, g)
                            if g < len(prev_proj):
                                prev_proj[g]()
                    elif j < TC - 1:
                        hook = (lambda g=g_idx: qk_piece(j + 1, xt_n, qt_n, g))
                    elif proj_hooks:
                        hook = proj_hooks[g_idx]  # chunk TC-2's projection
                    else:
                        hook = None
                    if "attn" in parts:
                        attn_group(j, hp, i, qt, yb, hook=hook)
                    elif hook is not None:
                        hook()
            if j < TC - 1:
                v_chunk(j + 1, xt_n)
                xt, qt = xt_n, qt_n
            flush()
            if "proj" in parts:
                if j == TC - 2:
                    proj_hooks = make_proj_hooks(j, yb)
                elif j == TC - 1:
                    # defer across the iteration boundary; build_kernel
                    # emits any leftovers after the last iteration
                    shared["proj_prev"] = make_proj_hooks(j, yb)
                else:
                    proj_block(j, yb)




_NC_CACHE = None
LAST_RESULT = None


def _prep_in_maps(x, w_qkv, w_proj):
    import ml_dtypes
    bf16 = ml_dtypes.bfloat16
    in_maps = []
    for m in range(N_CORES):
        b, g = m // 2, m % 2
        w_my = np.concatenate(
            [w_qkv[:, g * 512:(g + 1) * 512],
             w_qkv[:, 1024 + g * 512:1024 + (g + 1) * 512],
             w_qkv[:, 2048 + g * 512:2048 + (g + 1) * 512]], axis=1)
        in_maps.append({
            "x_bT": np.ascontiguousarray(x[b].T).astype(bf16),
            "w_qkv_my": np.ascontiguousarray(w_my).astype(bf16),
            "w_proj_my": np.ascontiguousarray(
                w_proj[g * 512:(g + 1) * 512, :]).astype(bf16),
        })
    return in_maps


def kernel(x, w_qkv, w_proj):
    global _NC_CACHE, LAST_RESULT
    x = np.asarray(x, dtype=np.float32)
    w_qkv = np.asarray(w_qkv, dtype=np.float32)
    w_proj = np.asarray(w_proj, dtype=np.float32)

    if _NC_CACHE is None:
        _NC_CACHE = build_kernel()
    nc = _NC_CACHE

    in_maps = _prep_in_maps(x, w_qkv, w_proj)
    res = run_bass_kernel_spmd(nc, in_maps, core_ids=list(range(N_CORES)))
    LAST_RESULT = res
    out = np.empty((B, T, D), dtype=np.float32)
    for b in range(B):
        out[b] = (res.results[2 * b]["out"].astype(np.float32)
                  + res.results[2 * b + 1]["out"].astype(np.float32))
    return out

